# revision 23
# baseline (speedup 1.0000x reference)
"""Trainium2 Bass kernel for nn_MultiHeadAttention (B=8, S=1024, D=1024, h=16).

Sharding: pure data-parallel over batch — each of the 8 NeuronCores computes
the full MHA for one batch element. No collectives.

Per-core pipeline (bf16 matmul operands, fp32 PSUM accumulation):
  1. Load Q,K,V row-major; PE-transpose 128x128 blocks into feature-major
     bf16 tiles (one 8-tile ring serves Q -> K -> V sequentially).
  2. Projections: stationary = weight block (bf16 -> fast weight load), one
     LDWEIGHTS feeds both 512-wide output chunks.
       Q/K: feature-major relu outputs; V: row-major, evicted head-major
       into "Vaug" tiles (per head 65 cols = 64 V-features + a ones column
       that accumulates the softmax denominator in the PV matmul).
  3. Attention per head-PAIR: even head on PE rows 0-63, odd head on rows
     64-127 — consecutive matmuls target disjoint row groups so weight
     loads overlap in-flight matmuls and the pair runs concurrently in the
     array. Scores are kept transposed (keys on partitions, queries free):
       P_T = exp(S_T / 32)  (ACT; scores are O(0.3): no max-subtraction)
     PV shares each Vaug stationary across both query chunks. The softmax
     division runs off a fast SBUF staging copy (psum ring never waits):
     DVE reciprocal on a DRAM-spread layout + broadcast + multiply.
  4. Output projection relu(Ot @ WO + bO); each Ot stationary feeds both
     output chunks.
"""
import os
from contextlib import ExitStack

import numpy as np

import concourse.bass as bass
import concourse.tile as tile
from concourse import mybir
from concourse.bass_utils import run_bass_kernel_spmd
from concourse.masks import make_identity

f32 = mybir.dt.float32
bf16 = mybir.dt.bfloat16
AF = mybir.ActivationFunctionType

S = 1024
D = 1024
H = 16
DK = 64
P = 128
NB = D // P  # 8 blocks
QC = 512
N_CORES = 8


def _split_wide_waits(nc, max_waits=1):
    """This walrus build rejects instructions carrying more than one
    semaphore wait; move excess waits onto NoOp carriers inserted before
    the offending instruction on the same engine."""
    for bb in nc.m.functions[0].blocks:
        idx = 0
        while idx < len(bb.instructions):
            ins = bb.instructions[idx]
            si = ins.sync_info
            if si is not None and si.on_wait and len(si.on_wait) > max_waits:
                waits = list(si.on_wait)
                rest, keep = waits[:-max_waits], waits[-max_waits:]
                for j in range(0, len(rest), max_waits):
                    nop = mybir.InstNoOp(
                        name=f"I-waitsplit-{nc.next_id()}",
                        engine=ins.engine,
                        ins=[],
                        outs=[],
                    )
                    nop.sync_info = mybir.SyncInfo(
                        on_wait=rest[j : j + max_waits], on_update=[]
                    )
                    nc.register_instruction(nop)
                    bb.instructions.insert(idx, nop)
                    idx += 1
                ins.sync_info = mybir.SyncInfo(
                    on_wait=keep, on_update=list(si.on_update)
                )
            idx += 1


def _build_nc(with_bv: bool, with_bo: bool):
    nc = bass.Bass("TRN2", target_bir_lowering=False, debug=False, num_devices=1)

    Qd = nc.dram_tensor("Q", [S, D], f32, kind="ExternalInput").ap()
    Kd = nc.dram_tensor("K", [S, D], f32, kind="ExternalInput").ap()
    Vd = nc.dram_tensor("V", [S, D], f32, kind="ExternalInput").ap()
    WQd = nc.dram_tensor("WQ", [D, D], f32, kind="ExternalInput").ap()
    WKd = nc.dram_tensor("WK", [D, D], f32, kind="ExternalInput").ap()
    WVd = nc.dram_tensor("WV", [D, D], f32, kind="ExternalInput").ap()
    WOd = nc.dram_tensor("WO", [D, D], f32, kind="ExternalInput").ap()
    bQd = nc.dram_tensor("bQ", [D], f32, kind="ExternalInput").ap()
    bKd = nc.dram_tensor("bK", [D], f32, kind="ExternalInput").ap()
    bVd = nc.dram_tensor("bV", [D], f32, kind="ExternalInput").ap()
    bOd = nc.dram_tensor("bO", [D], f32, kind="ExternalInput").ap()
    outd = nc.dram_tensor("out", [S, D], f32, kind="ExternalOutput").ap()

    with tile.TileContext(nc) as tc, ExitStack() as ctx:
        sb = ctx.enter_context(tc.tile_pool(name="sb", bufs=1))
        ps = ctx.enter_context(tc.tile_pool(name="ps", bufs=1, space="PSUM"))
        dramp = ctx.enter_context(tc.tile_pool(name="dram", bufs=1, space="DRAM"))

        # ---- constants -------------------------------------------------
        ident = sb.tile([P, P], f32, tag="ident", name="ident")
        make_identity(nc, ident)
        bqk = sb.tile([P, 2 * NB], f32, tag="bqk", name="bqk")
        nc.sync.dma_start(bqk[:, 0:NB], bQd.rearrange("(db p) -> p db", p=P))
        nc.sync.dma_start(bqk[:, NB : 2 * NB], bKd.rearrange("(db p) -> p db", p=P))
        if with_bv:
            bvb = sb.tile([P, D], f32, tag="bvb", name="bvb")
            nc.sync.dma_start(bvb, bVd[None, :].broadcast_to([P, D]))
        if with_bo:
            bob = sb.tile([P, D], f32, tag="bob", name="bob")
            nc.sync.dma_start(bob, bOd[None, :].broadcast_to([P, D]))

        def wload(Wd, kb, chunk):
            """Stream a [128, 512] f32 weight strip and cast to bf16."""
            wstage = sb.tile([P, QC], f32, tag="wstage", bufs=3, name="wstage")
            nc.sync.dma_start(
                wstage, Wd[kb * P : (kb + 1) * P, chunk * QC : (chunk + 1) * QC]
            )
            wb = sb.tile([P, QC], bf16, tag="wbf", bufs=18, name="wbf")
            nc.vector.tensor_copy(wb, wstage)
            return wb

        def load_transpose(Xd):
            """HBM row-major -> feature-major bf16 tiles xt[db] (128 x 1024)."""
            xt = [
                sb.tile([P, S], bf16, tag="xt", bufs=NB, name=f"xt{i}")
                for i in range(NB)
            ]
            for sblk in range(NB):
                xn = sb.tile([P, D], f32, tag="xn", bufs=3, name="xn")
                nc.sync.dma_start(xn, Xd[sblk * P : (sblk + 1) * P, :])
                for db in range(NB):
                    tp = ps.tile([P, 2, QC], f32, tag="big", bufs=3, name="tp")
                    nc.tensor.transpose(
                        tp[:, 0, 0:P], xn[:, db * P : (db + 1) * P], ident
                    )
                    nc.vector.tensor_copy(
                        xt[db][:, sblk * P : (sblk + 1) * P], tp[:, 0, 0:P]
                    )
            return xt

        def proj_feature_major(xt, Wd, bias_base, out_tag):
            """xpt[db] = relu(W[:,db-block].T @ xt + b[db-block]) -> bf16."""
            xpt = [
                sb.tile([P, S], bf16, tag=out_tag, bufs=NB, name=f"{out_tag}{i}")
                for i in range(NB)
            ]
            for half in range(2):
                strips = [wload(Wd, kb, half) for kb in range(NB)]
                for db in range(half * 4, half * 4 + 4):
                    acc = ps.tile([P, 2, QC], f32, tag="big", bufs=3, name="acc")
                    co = (db % 4) * P
                    for kb in range(NB):
                        wt = strips[kb][:, co : co + P]
                        first, last = kb == 0, kb == NB - 1
                        nc.tensor.matmul(
                            acc[:, 0, :], wt, xt[kb][:, 0:QC],
                            start=first, stop=last,
                        )
                        nc.tensor.matmul(
                            acc[:, 1, :], wt, xt[kb][:, QC:S],
                            start=first, stop=last,
                        )
                    nc.scalar.activation(
                        xpt[db].rearrange("p (c q) -> p c q", c=2),
                        acc,
                        AF.Relu,
                        bias=bqk[:, bias_base + db : bias_base + db + 1],
                    )
            return xpt

        # ---- Q / K ------------------------------------------------------
        with nc.named_scope("q_prep"):
            xt = load_transpose(Qd)
        with nc.named_scope("q_proj"):
            qpt = proj_feature_major(xt, WQd, 0, "qpt")
        with nc.named_scope("k_prep"):
            xt = load_transpose(Kd)
        with nc.named_scope("k_proj"):
            kpt = proj_feature_major(xt, WKd, NB, "kpt")

        # ---- V ----------------------------------------------------------
        with nc.named_scope("v_prep"):
            vt = load_transpose(Vd)
        with nc.named_scope("v_proj"):
            vaug = [
                sb.tile([P, H * 65], bf16, tag="vaug", bufs=NB, name=f"vaug{i}")
                for i in range(NB)
            ]
            for sblk in range(NB):
                nc.vector.memset(
                    vaug[sblk].rearrange("p (h c) -> p h c", c=65)[:, :, 64:65],
                    1.0,
                )
            wv = [[wload(WVd, kb, c) for c in range(2)] for kb in range(NB)]
            for sblk in range(NB):
                acc = [
                    ps.tile([P, QC], f32, tag="vp", bufs=2, name="vacc")
                    for _ in range(2)
                ]
                for kb in range(NB):
                    for c in range(2):
                        nc.tensor.matmul(
                            acc[c],
                            vt[kb][:, sblk * P : (sblk + 1) * P],
                            wv[kb][c],
                            start=(kb == 0),
                            stop=(kb == NB - 1),
                        )
                for c in range(2):
                    if with_bv:
                        nc.vector.tensor_add(
                            acc[c], acc[c], bvb[:, c * QC : (c + 1) * QC]
                        )
                    dst = vaug[sblk].rearrange("p (h c) -> p h c", c=65)[
                        :, c * 8 : (c + 1) * 8, 0:64
                    ]
                    nc.scalar.activation(
                        dst, acc[c].rearrange("p (h c) -> p h c", c=64), AF.Relu
                    )

        # ---- attention --------------------------------------------------
        ot = [
            sb.tile([P, S], bf16, tag="ot", bufs=NB, name=f"ot{i}")
            for i in range(NB)
        ]

        def emit_pv_tail(h, vp):
            dbq, off = h // 2, (h % 2) * DK
            for qc in range(2):
                qsl = slice(qc * QC, (qc + 1) * QC)
                stage = sb.tile([65, QC], f32, tag="stage", bufs=4, name="stage")
                nc.vector.tensor_copy(stage, vp[qc][0:65, :])
                scr = dramp.tile([1, QC], f32, tag="scr", bufs=6, name="scr")
                nc.gpsimd.dma_start(scr, stage[64:65, :])
                rcp = sb.tile([DK, NB], f32, tag="rcp", bufs=3, name="rcp")
                nc.gpsimd.dma_start(
                    rcp, scr.rearrange("o (a b) -> a (o b)", a=DK)
                )
                nc.vector.reciprocal(rcp, rcp)
                scr2 = dramp.tile([1, QC], f32, tag="scr2", bufs=6, name="scr2")
                nc.gpsimd.dma_start(
                    scr2.rearrange("o (a b) -> a (o b)", a=DK), rcp
                )
                bc = sb.tile([DK, QC], f32, tag="bc", bufs=3, name="bc")
                nc.gpsimd.dma_start(bc, scr2.broadcast_to([DK, QC]))
                if off == 0:
                    nc.vector.tensor_mul(ot[dbq][0:DK, qsl], stage[0:DK, :], bc)
                else:
                    tmp = sb.tile([DK, QC], bf16, tag="tmp", bufs=2, name="tmp")
                    nc.vector.tensor_mul(tmp, stage[0:DK, :], bc)
                    nc.gpsimd.dma_start(ot[dbq][DK:P, qsl], tmp)

        def gen_pv(h, pts):
            """PV for one head, both q-chunks (shared Vaug stationaries),
            yielded in 4 groups of 4 matmuls so the caller can interleave
            them between score/exp bursts; softmax division at the end."""
            vp = [
                ps.tile([P, QC], f32, tag="vp", bufs=2, name="vpacc")
                for _ in range(2)
            ]
            for g in range(4):
                for kb in (2 * g, 2 * g + 1):
                    for qc in range(2):
                        nc.tensor.matmul(
                            vp[qc][0:65, :],
                            vaug[kb][:, h * 65 : (h + 1) * 65],
                            pts[qc][:, kb, :],
                            start=(kb == 0),
                            stop=(kb == NB - 1),
                        )
                yield
            emit_pv_tail(h, vp)

        def emit_unit(d, qc, pv_gen):
            """Scores+exp for head pair (2d, 2d+1), one q-chunk, with the
            pending PV's matmul groups interleaved after each exp burst so
            neither PE nor ACT ever waits on the other. Even head on PE
            rows 0-63, odd head on rows 64-127 (disjoint row groups)."""
            qsl = slice(qc * QC, (qc + 1) * QC)
            ptA = sb.tile([P, NB, QC], bf16, tag="pt", bufs=8, name="ptA")
            ptB = sb.tile([P, NB, QC], bf16, tag="pt", bufs=8, name="ptB")
            for kb2 in range(NB // 2):
                spA = ps.tile([P, 2, QC], f32, tag="big", bufs=3, name="spA")
                spB = ps.tile([P, 2, QC], f32, tag="big", bufs=3, name="spB")
                for j in range(2):
                    kb = 2 * kb2 + j
                    ksl = slice(kb * P, (kb + 1) * P)
                    nc.tensor.matmul(
                        spA[:, j, :], kpt[d][0:DK, ksl], qpt[d][0:DK, qsl],
                        start=True, stop=True,
                    )
                    nc.tensor.matmul(
                        spB[:, j, :], kpt[d][DK:P, ksl], qpt[d][DK:P, qsl],
                        start=True, stop=True,
                    )
                nc.scalar.activation(
                    ptA[:, 2 * kb2 : 2 * kb2 + 2, :], spA, AF.Exp, scale=0.03125
                )
                nc.scalar.activation(
                    ptB[:, 2 * kb2 : 2 * kb2 + 2, :], spB, AF.Exp, scale=0.03125
                )
                if pv_gen is not None:
                    next(pv_gen, None)
            return ptA, ptB

        with nc.named_scope("attention"):
            # software-pipelined over head pairs; the previous pair's PV
            # matmuls ride inside the current scores unit:
            #   S(d,0)+PV_A(d-1), S(d,1)+PV_B(d-1), S(d+1,0)+PV_A(d), ...
            pend = {}
            prev = None
            for d in range(NB):
                g = (
                    gen_pv(2 * prev, [pend[(prev, 0)][0], pend[(prev, 1)][0]])
                    if prev is not None
                    else None
                )
                a0, b0 = emit_unit(d, 0, g)
                if g is not None:
                    for _ in g:
                        pass
                g = (
                    gen_pv(
                        2 * prev + 1, [pend[(prev, 0)][1], pend[(prev, 1)][1]]
                    )
                    if prev is not None
                    else None
                )
                a1, b1 = emit_unit(d, 1, g)
                if g is not None:
                    for _ in g:
                        pass
                    del pend[(prev, 0)], pend[(prev, 1)]
                pend[(d, 0)] = (a0, b0)
                pend[(d, 1)] = (a1, b1)
                prev = d
            for _ in gen_pv(2 * prev, [pend[(prev, 0)][0], pend[(prev, 1)][0]]):
                pass
            for _ in gen_pv(
                2 * prev + 1, [pend[(prev, 0)][1], pend[(prev, 1)][1]]
            ):
                pass

        # ---- output projection -----------------------------------------
        with nc.named_scope("o_proj"):
            wo = [[wload(WOd, db, c) for c in range(2)] for db in range(NB)]
            for sblk in range(NB):
                acc = [
                    ps.tile([P, QC], f32, tag="vp", bufs=2, name="oacc")
                    for _ in range(2)
                ]
                for db in range(NB):
                    for c in range(2):
                        nc.tensor.matmul(
                            acc[c],
                            ot[db][:, sblk * P : (sblk + 1) * P],
                            wo[db][c],
                            start=(db == 0),
                            stop=(db == NB - 1),
                        )
                for c in range(2):
                    if with_bo:
                        nc.vector.tensor_add(
                            acc[c], acc[c], bob[:, c * QC : (c + 1) * QC]
                        )
                    o = sb.tile([P, QC], f32, tag="obuf", bufs=3, name="obuf")
                    nc.scalar.activation(o, acc[c], AF.Relu)
                    nc.sync.dma_start(
                        outd[sblk * P : (sblk + 1) * P, c * QC : (c + 1) * QC], o
                    )

    _split_wide_waits(nc)
    return nc


_NC_CACHE = {}


def kernel(Q, K, V, WQ, bQ, WK, bK, WV, bV, WO, bO, h):
    Q, K, V = (np.ascontiguousarray(np.asarray(x, np.float32)) for x in (Q, K, V))
    WQ, WK, WV, WO = (
        np.ascontiguousarray(np.asarray(x, np.float32)) for x in (WQ, WK, WV, WO)
    )
    bQ, bK, bV, bO = (
        np.ascontiguousarray(np.asarray(x, np.float32)) for x in (bQ, bK, bV, bO)
    )
    h = int(np.asarray(h))
    assert h == H, f"kernel specialized for h=16, got {h}"
    B = Q.shape[0]
    assert Q.shape == (B, S, D) and B == N_CORES

    key = (bool(np.any(bV)), bool(np.any(bO)))
    if key not in _NC_CACHE:
        _NC_CACHE[key] = _build_nc(*key)
    nc = _NC_CACHE[key]

    in_maps = [
        {
            "Q": Q[b], "K": K[b], "V": V[b],
            "WQ": WQ, "WK": WK, "WV": WV, "WO": WO,
            "bQ": bQ, "bK": bK, "bV": bV, "bO": bO,
        }
        for b in range(B)
    ]
    trace = os.environ.get("BASS_MHA_TRACE") == "1"
    res = run_bass_kernel_spmd(
        nc, in_maps, core_ids=list(range(N_CORES)), trace=trace
    )
    if trace:
        kernel.last_results = res
    return np.stack([res.results[b]["out"] for b in range(B)], axis=0)


# revision 24
# speedup vs baseline: 1.0460x; 1.0460x over previous
"""Trainium2 Bass kernel for nn_MultiHeadAttention (B=8, S=1024, D=1024, h=16).

Sharding: pure data-parallel over batch — each of the 8 NeuronCores computes
the full MHA for one batch element. No collectives.

Per-core pipeline (bf16 matmul operands, fp32 PSUM accumulation):
  1. Load Q,K,V row-major; PE-transpose 128x128 blocks into feature-major
     bf16 tiles (one 8-tile ring serves Q -> K -> V sequentially).
  2. Projections: stationary = weight block (bf16 -> fast weight load), one
     LDWEIGHTS feeds both 512-wide output chunks.
       Q/K: feature-major relu outputs; V: row-major, evicted head-major
       into "Vaug" tiles (per head 65 cols = 64 V-features + a ones column
       that accumulates the softmax denominator in the PV matmul).
  3. Attention per head-PAIR: even head on PE rows 0-63, odd head on rows
     64-127 — consecutive matmuls target disjoint row groups so weight
     loads overlap in-flight matmuls and the pair runs concurrently in the
     array. Scores are kept transposed (keys on partitions, queries free):
       P_T = exp(S_T / 32)  (ACT; scores are O(0.3): no max-subtraction)
     PV shares each Vaug stationary across both query chunks. The softmax
     division runs off a fast SBUF staging copy (psum ring never waits):
     DVE reciprocal on a DRAM-spread layout + broadcast + multiply.
  4. Output projection relu(Ot @ WO + bO); each Ot stationary feeds both
     output chunks.
"""
import os
from contextlib import ExitStack

import numpy as np

import concourse.bass as bass
import concourse.tile as tile
from concourse import mybir
from concourse.bass_utils import run_bass_kernel_spmd
from concourse.masks import make_identity

f32 = mybir.dt.float32
bf16 = mybir.dt.bfloat16
AF = mybir.ActivationFunctionType

S = 1024
D = 1024
H = 16
DK = 64
P = 128
NB = D // P  # 8 blocks
QC = 512
N_CORES = 8


def _split_wide_waits(nc, max_waits=1):
    """This walrus build rejects instructions carrying more than one
    semaphore wait; move excess waits onto NoOp carriers inserted before
    the offending instruction on the same engine."""
    for bb in nc.m.functions[0].blocks:
        idx = 0
        while idx < len(bb.instructions):
            ins = bb.instructions[idx]
            si = ins.sync_info
            if si is not None and si.on_wait and len(si.on_wait) > max_waits:
                waits = list(si.on_wait)
                rest, keep = waits[:-max_waits], waits[-max_waits:]
                for j in range(0, len(rest), max_waits):
                    nop = mybir.InstNoOp(
                        name=f"I-waitsplit-{nc.next_id()}",
                        engine=ins.engine,
                        ins=[],
                        outs=[],
                    )
                    nop.sync_info = mybir.SyncInfo(
                        on_wait=rest[j : j + max_waits], on_update=[]
                    )
                    nc.register_instruction(nop)
                    bb.instructions.insert(idx, nop)
                    idx += 1
                ins.sync_info = mybir.SyncInfo(
                    on_wait=keep, on_update=list(si.on_update)
                )
            idx += 1


def _build_nc(with_bv: bool, with_bo: bool):
    nc = bass.Bass("TRN2", target_bir_lowering=False, debug=False, num_devices=1)

    Qd = nc.dram_tensor("Q", [S, D], f32, kind="ExternalInput").ap()
    Kd = nc.dram_tensor("K", [S, D], f32, kind="ExternalInput").ap()
    Vd = nc.dram_tensor("V", [S, D], f32, kind="ExternalInput").ap()
    WQd = nc.dram_tensor("WQ", [D, D], f32, kind="ExternalInput").ap()
    WKd = nc.dram_tensor("WK", [D, D], f32, kind="ExternalInput").ap()
    WVd = nc.dram_tensor("WV", [D, D], f32, kind="ExternalInput").ap()
    WOd = nc.dram_tensor("WO", [D, D], f32, kind="ExternalInput").ap()
    bQd = nc.dram_tensor("bQ", [D], f32, kind="ExternalInput").ap()
    bKd = nc.dram_tensor("bK", [D], f32, kind="ExternalInput").ap()
    bVd = nc.dram_tensor("bV", [D], f32, kind="ExternalInput").ap()
    bOd = nc.dram_tensor("bO", [D], f32, kind="ExternalInput").ap()
    outd = nc.dram_tensor("out", [S, D], f32, kind="ExternalOutput").ap()

    with tile.TileContext(nc) as tc, ExitStack() as ctx:
        sb = ctx.enter_context(tc.tile_pool(name="sb", bufs=1))
        ps = ctx.enter_context(tc.tile_pool(name="ps", bufs=1, space="PSUM"))
        dramp = ctx.enter_context(tc.tile_pool(name="dram", bufs=1, space="DRAM"))

        # ---- constants -------------------------------------------------
        ident = sb.tile([P, P], f32, tag="ident", name="ident")
        make_identity(nc, ident)
        bqk = sb.tile([P, 2 * NB], f32, tag="bqk", name="bqk")
        nc.sync.dma_start(bqk[:, 0:NB], bQd.rearrange("(db p) -> p db", p=P))
        nc.sync.dma_start(bqk[:, NB : 2 * NB], bKd.rearrange("(db p) -> p db", p=P))
        if with_bv:
            bvb = sb.tile([P, D], f32, tag="bvb", name="bvb")
            nc.sync.dma_start(bvb, bVd[None, :].broadcast_to([P, D]))
        if with_bo:
            bob = sb.tile([P, D], f32, tag="bob", name="bob")
            nc.sync.dma_start(bob, bOd[None, :].broadcast_to([P, D]))

        def wload(Wd, kb, chunk):
            """Stream a [128, 512] f32 weight strip and cast to bf16."""
            wstage = sb.tile([P, QC], f32, tag="wstage", bufs=3, name="wstage")
            nc.sync.dma_start(
                wstage, Wd[kb * P : (kb + 1) * P, chunk * QC : (chunk + 1) * QC]
            )
            wb = sb.tile([P, QC], bf16, tag="wbf", bufs=18, name="wbf")
            nc.vector.tensor_copy(wb, wstage)
            return wb

        def load_transpose(Xd):
            """HBM row-major -> feature-major bf16 tiles xt[db] (128 x 1024)."""
            xt = [
                sb.tile([P, S], bf16, tag="xt", bufs=NB, name=f"xt{i}")
                for i in range(NB)
            ]
            for sblk in range(NB):
                xn = sb.tile([P, D], f32, tag="xn", bufs=3, name="xn")
                nc.sync.dma_start(xn, Xd[sblk * P : (sblk + 1) * P, :])
                for db in range(NB):
                    tp = ps.tile([P, 2, QC], f32, tag="big", bufs=3, name="tp")
                    nc.tensor.transpose(
                        tp[:, 0, 0:P], xn[:, db * P : (db + 1) * P], ident
                    )
                    dst = xt[db][:, sblk * P : (sblk + 1) * P]
                    if (sblk + db) % 2 == 0:
                        nc.vector.tensor_copy(dst, tp[:, 0, 0:P])
                    else:
                        nc.scalar.activation(dst, tp[:, 0, 0:P], AF.Copy)
            return xt

        def proj_feature_major(xt, Wd, bias_base, out_tag):
            """xpt[db] = relu(W[:,db-block].T @ xt + b[db-block]) -> bf16."""
            xpt = [
                sb.tile([P, S], bf16, tag=out_tag, bufs=NB, name=f"{out_tag}{i}")
                for i in range(NB)
            ]
            for half in range(2):
                strips = [wload(Wd, kb, half) for kb in range(NB)]
                for db in range(half * 4, half * 4 + 4):
                    acc = ps.tile([P, 2, QC], f32, tag="big", bufs=3, name="acc")
                    co = (db % 4) * P
                    for kb in range(NB):
                        wt = strips[kb][:, co : co + P]
                        first, last = kb == 0, kb == NB - 1
                        nc.tensor.matmul(
                            acc[:, 0, :], wt, xt[kb][:, 0:QC],
                            start=first, stop=last,
                        )
                        nc.tensor.matmul(
                            acc[:, 1, :], wt, xt[kb][:, QC:S],
                            start=first, stop=last,
                        )
                    nc.scalar.activation(
                        xpt[db].rearrange("p (c q) -> p c q", c=2),
                        acc,
                        AF.Relu,
                        bias=bqk[:, bias_base + db : bias_base + db + 1],
                    )
            return xpt

        # ---- Q / K ------------------------------------------------------
        with nc.named_scope("q_prep"):
            xt = load_transpose(Qd)
        with nc.named_scope("q_proj"):
            qpt = proj_feature_major(xt, WQd, 0, "qpt")
        with nc.named_scope("k_prep"):
            xt = load_transpose(Kd)
        with nc.named_scope("k_proj"):
            kpt = proj_feature_major(xt, WKd, NB, "kpt")

        # ---- V ----------------------------------------------------------
        with nc.named_scope("v_prep"):
            vt = load_transpose(Vd)
        with nc.named_scope("v_proj"):
            vaug = [
                sb.tile([P, H * 65], bf16, tag="vaug", bufs=NB, name=f"vaug{i}")
                for i in range(NB)
            ]
            for sblk in range(NB):
                nc.vector.memset(
                    vaug[sblk].rearrange("p (h c) -> p h c", c=65)[:, :, 64:65],
                    1.0,
                )
            wv = [[wload(WVd, kb, c) for c in range(2)] for kb in range(NB)]
            for sblk in range(NB):
                acc = [
                    ps.tile([P, QC], f32, tag="vp", bufs=2, name="vacc")
                    for _ in range(2)
                ]
                for kb in range(NB):
                    for c in range(2):
                        nc.tensor.matmul(
                            acc[c],
                            vt[kb][:, sblk * P : (sblk + 1) * P],
                            wv[kb][c],
                            start=(kb == 0),
                            stop=(kb == NB - 1),
                        )
                for c in range(2):
                    if with_bv:
                        nc.vector.tensor_add(
                            acc[c], acc[c], bvb[:, c * QC : (c + 1) * QC]
                        )
                    dst = vaug[sblk].rearrange("p (h c) -> p h c", c=65)[
                        :, c * 8 : (c + 1) * 8, 0:64
                    ]
                    nc.scalar.activation(
                        dst, acc[c].rearrange("p (h c) -> p h c", c=64), AF.Relu
                    )

        # ---- attention --------------------------------------------------
        ot = [
            sb.tile([P, S], bf16, tag="ot", bufs=NB, name=f"ot{i}")
            for i in range(NB)
        ]

        def emit_pv_tail(h, vp):
            dbq, off = h // 2, (h % 2) * DK
            for qc in range(2):
                qsl = slice(qc * QC, (qc + 1) * QC)
                stage = sb.tile([65, QC], f32, tag="stage", bufs=4, name="stage")
                nc.vector.tensor_copy(stage, vp[qc][0:65, :])
                scr = dramp.tile([1, QC], f32, tag="scr", bufs=6, name="scr")
                nc.gpsimd.dma_start(scr, stage[64:65, :])
                rcp = sb.tile([DK, NB], f32, tag="rcp", bufs=3, name="rcp")
                nc.gpsimd.dma_start(
                    rcp, scr.rearrange("o (a b) -> a (o b)", a=DK)
                )
                nc.vector.reciprocal(rcp, rcp)
                scr2 = dramp.tile([1, QC], f32, tag="scr2", bufs=6, name="scr2")
                nc.gpsimd.dma_start(
                    scr2.rearrange("o (a b) -> a (o b)", a=DK), rcp
                )
                bc = sb.tile([DK, QC], f32, tag="bc", bufs=3, name="bc")
                nc.gpsimd.dma_start(bc, scr2.broadcast_to([DK, QC]))
                if off == 0:
                    nc.vector.tensor_mul(ot[dbq][0:DK, qsl], stage[0:DK, :], bc)
                else:
                    tmp = sb.tile([DK, QC], bf16, tag="tmp", bufs=2, name="tmp")
                    nc.vector.tensor_mul(tmp, stage[0:DK, :], bc)
                    nc.gpsimd.dma_start(ot[dbq][DK:P, qsl], tmp)

        def gen_pv(h, pts):
            """PV for one head, both q-chunks (shared Vaug stationaries),
            yielded in 4 groups of 4 matmuls so the caller can interleave
            them between score/exp bursts; softmax division at the end."""
            vp = [
                ps.tile([P, QC], f32, tag="vp", bufs=2, name="vpacc")
                for _ in range(2)
            ]
            for g in range(4):
                for kb in (2 * g, 2 * g + 1):
                    for qc in range(2):
                        nc.tensor.matmul(
                            vp[qc][0:65, :],
                            vaug[kb][:, h * 65 : (h + 1) * 65],
                            pts[qc][:, kb, :],
                            start=(kb == 0),
                            stop=(kb == NB - 1),
                        )
                yield
            emit_pv_tail(h, vp)

        def emit_unit(d, qc, pv_gen):
            """Scores+exp for head pair (2d, 2d+1), one q-chunk, with the
            pending PV's matmul groups interleaved after each exp burst so
            neither PE nor ACT ever waits on the other. Even head on PE
            rows 0-63, odd head on rows 64-127 (disjoint row groups)."""
            qsl = slice(qc * QC, (qc + 1) * QC)
            ptA = sb.tile([P, NB, QC], bf16, tag="pt", bufs=8, name="ptA")
            ptB = sb.tile([P, NB, QC], bf16, tag="pt", bufs=8, name="ptB")
            for kb2 in range(NB // 2):
                spA = ps.tile([P, 2, QC], f32, tag="big", bufs=3, name="spA")
                spB = ps.tile([P, 2, QC], f32, tag="big", bufs=3, name="spB")
                for j in range(2):
                    kb = 2 * kb2 + j
                    ksl = slice(kb * P, (kb + 1) * P)
                    nc.tensor.matmul(
                        spA[:, j, :], kpt[d][0:DK, ksl], qpt[d][0:DK, qsl],
                        start=True, stop=True,
                    )
                    nc.tensor.matmul(
                        spB[:, j, :], kpt[d][DK:P, ksl], qpt[d][DK:P, qsl],
                        start=True, stop=True,
                    )
                nc.scalar.activation(
                    ptA[:, 2 * kb2 : 2 * kb2 + 2, :], spA, AF.Exp, scale=0.03125
                )
                nc.scalar.activation(
                    ptB[:, 2 * kb2 : 2 * kb2 + 2, :], spB, AF.Exp, scale=0.03125
                )
                if pv_gen is not None:
                    next(pv_gen, None)
            return ptA, ptB

        with nc.named_scope("attention"):
            # software-pipelined over head pairs; the previous pair's PV
            # matmuls ride inside the current scores unit:
            #   S(d,0)+PV_A(d-1), S(d,1)+PV_B(d-1), S(d+1,0)+PV_A(d), ...
            pend = {}
            prev = None
            for d in range(NB):
                g = (
                    gen_pv(2 * prev, [pend[(prev, 0)][0], pend[(prev, 1)][0]])
                    if prev is not None
                    else None
                )
                a0, b0 = emit_unit(d, 0, g)
                if g is not None:
                    for _ in g:
                        pass
                g = (
                    gen_pv(
                        2 * prev + 1, [pend[(prev, 0)][1], pend[(prev, 1)][1]]
                    )
                    if prev is not None
                    else None
                )
                a1, b1 = emit_unit(d, 1, g)
                if g is not None:
                    for _ in g:
                        pass
                    del pend[(prev, 0)], pend[(prev, 1)]
                pend[(d, 0)] = (a0, b0)
                pend[(d, 1)] = (a1, b1)
                prev = d
            for _ in gen_pv(2 * prev, [pend[(prev, 0)][0], pend[(prev, 1)][0]]):
                pass
            for _ in gen_pv(
                2 * prev + 1, [pend[(prev, 0)][1], pend[(prev, 1)][1]]
            ):
                pass

        # ---- output projection -----------------------------------------
        with nc.named_scope("o_proj"):
            wo = [[wload(WOd, db, c) for c in range(2)] for db in range(NB)]
            for sblk in range(NB):
                bigacc = ps.tile([P, 2, QC], f32, tag="big", bufs=3, name="oacc")
                acc = [bigacc[:, 0, :], bigacc[:, 1, :]]
                for db in range(NB):
                    for c in range(2):
                        nc.tensor.matmul(
                            acc[c],
                            ot[db][:, sblk * P : (sblk + 1) * P],
                            wo[db][c],
                            start=(db == 0),
                            stop=(db == NB - 1),
                        )
                for c in range(2):
                    if with_bo:
                        nc.vector.tensor_add(
                            acc[c], acc[c], bob[:, c * QC : (c + 1) * QC]
                        )
                    o = sb.tile([P, QC], f32, tag="obuf", bufs=3, name="obuf")
                    nc.scalar.activation(o, acc[c], AF.Relu)
                    nc.sync.dma_start(
                        outd[sblk * P : (sblk + 1) * P, c * QC : (c + 1) * QC], o
                    )

    _split_wide_waits(nc)
    return nc


_NC_CACHE = {}


def kernel(Q, K, V, WQ, bQ, WK, bK, WV, bV, WO, bO, h):
    Q, K, V = (np.ascontiguousarray(np.asarray(x, np.float32)) for x in (Q, K, V))
    WQ, WK, WV, WO = (
        np.ascontiguousarray(np.asarray(x, np.float32)) for x in (WQ, WK, WV, WO)
    )
    bQ, bK, bV, bO = (
        np.ascontiguousarray(np.asarray(x, np.float32)) for x in (bQ, bK, bV, bO)
    )
    h = int(np.asarray(h))
    assert h == H, f"kernel specialized for h=16, got {h}"
    B = Q.shape[0]
    assert Q.shape == (B, S, D) and B == N_CORES

    key = (bool(np.any(bV)), bool(np.any(bO)))
    if key not in _NC_CACHE:
        _NC_CACHE[key] = _build_nc(*key)
    nc = _NC_CACHE[key]

    in_maps = [
        {
            "Q": Q[b], "K": K[b], "V": V[b],
            "WQ": WQ, "WK": WK, "WV": WV, "WO": WO,
            "bQ": bQ, "bK": bK, "bV": bV, "bO": bO,
        }
        for b in range(B)
    ]
    trace = os.environ.get("BASS_MHA_TRACE") == "1"
    res = run_bass_kernel_spmd(
        nc, in_maps, core_ids=list(range(N_CORES)), trace=trace
    )
    if trace:
        kernel.last_results = res
    return np.stack([res.results[b]["out"] for b in range(B)], axis=0)


# revision 26
# speedup vs baseline: 1.1135x; 1.0646x over previous
"""Trainium2 Bass kernel for nn_MultiHeadAttention (B=8, S=1024, D=1024, h=16).

Sharding: pure data-parallel over batch — each of the 8 NeuronCores computes
the full MHA for one batch element. No collectives.

Per-core pipeline (bf16 matmul operands, fp32 PSUM accumulation):
  1. Load Q,K,V row-major; PE-transpose 128x128 blocks into feature-major
     bf16 tiles (one 8-tile ring serves Q -> K -> V sequentially).
  2. Projections: stationary = weight block (bf16 -> fast weight load), one
     LDWEIGHTS feeds both 512-wide output chunks.
       Q/K: feature-major relu outputs; V: row-major, evicted head-major
       into "Vaug" tiles (per head 65 cols = 64 V-features + a ones column
       that accumulates the softmax denominator in the PV matmul).
  3. Attention per head-PAIR: even head on PE rows 0-63, odd head on rows
     64-127 — consecutive matmuls target disjoint row groups so weight
     loads overlap in-flight matmuls and the pair runs concurrently in the
     array. Scores are kept transposed (keys on partitions, queries free):
       P_T = exp(S_T / 32)  (ACT; scores are O(0.3): no max-subtraction)
     PV shares each Vaug stationary across both query chunks. The softmax
     division runs off a fast SBUF staging copy (psum ring never waits):
     DVE reciprocal on a DRAM-spread layout + broadcast + multiply.
  4. Output projection relu(Ot @ WO + bO); each Ot stationary feeds both
     output chunks.
"""
import os
from contextlib import ExitStack

import numpy as np

import concourse.bass as bass
import concourse.tile as tile
from concourse import mybir
from concourse.bass_utils import run_bass_kernel_spmd
from concourse.masks import make_identity

f32 = mybir.dt.float32
bf16 = mybir.dt.bfloat16
AF = mybir.ActivationFunctionType

S = 1024
D = 1024
H = 16
DK = 64
P = 128
NB = D // P  # 8 blocks
QC = 512
N_CORES = 8


def _split_wide_waits(nc, max_waits=1):
    """This walrus build rejects instructions carrying more than one
    semaphore wait; move excess waits onto NoOp carriers inserted before
    the offending instruction on the same engine."""
    for bb in nc.m.functions[0].blocks:
        idx = 0
        while idx < len(bb.instructions):
            ins = bb.instructions[idx]
            si = ins.sync_info
            if si is not None and si.on_wait and len(si.on_wait) > max_waits:
                waits = list(si.on_wait)
                rest, keep = waits[:-max_waits], waits[-max_waits:]
                for j in range(0, len(rest), max_waits):
                    nop = mybir.InstNoOp(
                        name=f"I-waitsplit-{nc.next_id()}",
                        engine=ins.engine,
                        ins=[],
                        outs=[],
                    )
                    nop.sync_info = mybir.SyncInfo(
                        on_wait=rest[j : j + max_waits], on_update=[]
                    )
                    nc.register_instruction(nop)
                    bb.instructions.insert(idx, nop)
                    idx += 1
                ins.sync_info = mybir.SyncInfo(
                    on_wait=keep, on_update=list(si.on_update)
                )
            idx += 1


def _build_nc(with_bv: bool, with_bo: bool):
    nc = bass.Bass("TRN2", target_bir_lowering=False, debug=False, num_devices=1)

    Qd = nc.dram_tensor("Q", [S, D], f32, kind="ExternalInput").ap()
    Kd = nc.dram_tensor("K", [S, D], f32, kind="ExternalInput").ap()
    Vd = nc.dram_tensor("V", [S, D], f32, kind="ExternalInput").ap()
    WQd = nc.dram_tensor("WQ", [D, D], f32, kind="ExternalInput").ap()
    WKd = nc.dram_tensor("WK", [D, D], f32, kind="ExternalInput").ap()
    WVd = nc.dram_tensor("WV", [D, D], f32, kind="ExternalInput").ap()
    WOd = nc.dram_tensor("WO", [D, D], f32, kind="ExternalInput").ap()
    bQd = nc.dram_tensor("bQ", [D], f32, kind="ExternalInput").ap()
    bKd = nc.dram_tensor("bK", [D], f32, kind="ExternalInput").ap()
    bVd = nc.dram_tensor("bV", [D], f32, kind="ExternalInput").ap()
    bOd = nc.dram_tensor("bO", [D], f32, kind="ExternalInput").ap()
    outd = nc.dram_tensor("out", [S, D], f32, kind="ExternalOutput").ap()

    with tile.TileContext(nc) as tc, ExitStack() as ctx:
        sb = ctx.enter_context(tc.tile_pool(name="sb", bufs=1))
        ps = ctx.enter_context(tc.tile_pool(name="ps", bufs=1, space="PSUM"))
        dramp = ctx.enter_context(tc.tile_pool(name="dram", bufs=1, space="DRAM"))

        # ---- constants -------------------------------------------------
        ident = sb.tile([P, P], f32, tag="ident", name="ident")
        make_identity(nc, ident)
        identb = sb.tile([P, P], bf16, tag="identb", name="identb")
        nc.vector.tensor_copy(identb, ident)
        bqk = sb.tile([P, 2 * NB], f32, tag="bqk", name="bqk")
        nc.sync.dma_start(bqk[:, 0:NB], bQd.rearrange("(db p) -> p db", p=P))
        nc.sync.dma_start(bqk[:, NB : 2 * NB], bKd.rearrange("(db p) -> p db", p=P))
        if with_bv:
            bvb = sb.tile([P, D], f32, tag="bvb", name="bvb")
            nc.sync.dma_start(bvb, bVd[None, :].broadcast_to([P, D]))
        if with_bo:
            bob = sb.tile([P, D], f32, tag="bob", name="bob")
            nc.sync.dma_start(bob, bOd[None, :].broadcast_to([P, D]))

        def wload(Wd, kb, chunk):
            """Stream a [128, 512] f32 weight strip and cast to bf16."""
            wstage = sb.tile([P, QC], f32, tag="wstage", bufs=3, name="wstage")
            nc.sync.dma_start(
                wstage, Wd[kb * P : (kb + 1) * P, chunk * QC : (chunk + 1) * QC]
            )
            wb = sb.tile([P, QC], bf16, tag="wbf", bufs=18, name="wbf")
            nc.vector.tensor_copy(wb, wstage)
            return wb

        def load_transpose(Xd):
            """HBM row-major -> feature-major bf16 tiles xt[db] (128 x 1024)."""
            xt = [
                sb.tile([P, S], bf16, tag="xt", bufs=NB, name=f"xt{i}")
                for i in range(NB)
            ]
            for sblk in range(NB):
                xn = sb.tile([P, D], f32, tag="xn", bufs=2, name="xn")
                nc.sync.dma_start(xn, Xd[sblk * P : (sblk + 1) * P, :])
                xnb = sb.tile([P, D], bf16, tag="xnb", bufs=2, name="xnb")
                nc.vector.tensor_copy(xnb, xn)
                for db in range(NB):
                    tp = ps.tile([P, 2, QC], f32, tag="big", bufs=3, name="tp")
                    tpb = tp[:, 0, 0:P].bitcast(bf16)[:, 0:P]
                    nc.tensor.transpose(
                        tpb, xnb[:, db * P : (db + 1) * P], identb
                    )
                    dst = xt[db][:, sblk * P : (sblk + 1) * P]
                    if (sblk + db) % 2 == 0:
                        nc.vector.tensor_copy(dst, tpb)
                    else:
                        nc.scalar.activation(dst, tpb, AF.Copy)
            return xt

        def proj_feature_major(xt, Wd, bias_base, out_tag):
            """xpt[db] = relu(W[:,db-block].T @ xt + b[db-block]) -> bf16."""
            xpt = [
                sb.tile([P, S], bf16, tag=out_tag, bufs=NB, name=f"{out_tag}{i}")
                for i in range(NB)
            ]
            for half in range(2):
                strips = [wload(Wd, kb, half) for kb in range(NB)]
                for db in range(half * 4, half * 4 + 4):
                    acc = ps.tile([P, 2, QC], f32, tag="big", bufs=3, name="acc")
                    co = (db % 4) * P
                    for kb in range(NB):
                        wt = strips[kb][:, co : co + P]
                        first, last = kb == 0, kb == NB - 1
                        nc.tensor.matmul(
                            acc[:, 0, :], wt, xt[kb][:, 0:QC],
                            start=first, stop=last,
                        )
                        nc.tensor.matmul(
                            acc[:, 1, :], wt, xt[kb][:, QC:S],
                            start=first, stop=last,
                        )
                    nc.scalar.activation(
                        xpt[db].rearrange("p (c q) -> p c q", c=2),
                        acc,
                        AF.Relu,
                        bias=bqk[:, bias_base + db : bias_base + db + 1],
                    )
            return xpt

        # ---- Q / K ------------------------------------------------------
        with nc.named_scope("q_prep"):
            xt = load_transpose(Qd)
        with nc.named_scope("q_proj"):
            qpt = proj_feature_major(xt, WQd, 0, "qpt")
        with nc.named_scope("k_prep"):
            xt = load_transpose(Kd)
        with nc.named_scope("k_proj"):
            kpt = proj_feature_major(xt, WKd, NB, "kpt")

        # ---- V ----------------------------------------------------------
        with nc.named_scope("v_prep"):
            vt = load_transpose(Vd)
        with nc.named_scope("v_proj"):
            vaug = [
                sb.tile([P, H * 65], bf16, tag="vaug", bufs=NB, name=f"vaug{i}")
                for i in range(NB)
            ]
            for sblk in range(NB):
                nc.vector.memset(
                    vaug[sblk].rearrange("p (h c) -> p h c", c=65)[:, :, 64:65],
                    1.0,
                )
            wv = [[wload(WVd, kb, c) for c in range(2)] for kb in range(NB)]
            for sblk in range(NB):
                acc = [
                    ps.tile([P, QC], f32, tag="vp", bufs=2, name="vacc")
                    for _ in range(2)
                ]
                for kb in range(NB):
                    for c in range(2):
                        nc.tensor.matmul(
                            acc[c],
                            vt[kb][:, sblk * P : (sblk + 1) * P],
                            wv[kb][c],
                            start=(kb == 0),
                            stop=(kb == NB - 1),
                        )
                for c in range(2):
                    if with_bv:
                        nc.vector.tensor_add(
                            acc[c], acc[c], bvb[:, c * QC : (c + 1) * QC]
                        )
                    dst = vaug[sblk].rearrange("p (h c) -> p h c", c=65)[
                        :, c * 8 : (c + 1) * 8, 0:64
                    ]
                    nc.scalar.activation(
                        dst, acc[c].rearrange("p (h c) -> p h c", c=64), AF.Relu
                    )

        # ---- attention --------------------------------------------------
        ot = [
            sb.tile([P, S], bf16, tag="ot", bufs=NB, name=f"ot{i}")
            for i in range(NB)
        ]

        def emit_pv_tail(h, vp):
            dbq, off = h // 2, (h % 2) * DK
            for qc in range(2):
                qsl = slice(qc * QC, (qc + 1) * QC)
                stage = sb.tile([65, QC], f32, tag="stage", bufs=4, name="stage")
                nc.vector.tensor_copy(stage, vp[qc][0:65, :])
                scr = dramp.tile([1, QC], f32, tag="scr", bufs=6, name="scr")
                nc.gpsimd.dma_start(scr, stage[64:65, :])
                rcp = sb.tile([DK, NB], f32, tag="rcp", bufs=3, name="rcp")
                nc.gpsimd.dma_start(
                    rcp, scr.rearrange("o (a b) -> a (o b)", a=DK)
                )
                nc.vector.reciprocal(rcp, rcp)
                scr2 = dramp.tile([1, QC], f32, tag="scr2", bufs=6, name="scr2")
                nc.gpsimd.dma_start(
                    scr2.rearrange("o (a b) -> a (o b)", a=DK), rcp
                )
                bc = sb.tile([DK, QC], f32, tag="bc", bufs=3, name="bc")
                nc.gpsimd.dma_start(bc, scr2.broadcast_to([DK, QC]))
                if off == 0:
                    nc.vector.tensor_mul(ot[dbq][0:DK, qsl], stage[0:DK, :], bc)
                else:
                    tmp = sb.tile([DK, QC], bf16, tag="tmp", bufs=2, name="tmp")
                    nc.vector.tensor_mul(tmp, stage[0:DK, :], bc)
                    nc.gpsimd.dma_start(ot[dbq][DK:P, qsl], tmp)

        def gen_pv(h, pts):
            """PV for one head, both q-chunks (shared Vaug stationaries),
            yielded in 4 groups of 4 matmuls so the caller can interleave
            them between score/exp bursts; softmax division at the end."""
            vp = [
                ps.tile([P, QC], f32, tag="vp", bufs=2, name="vpacc")
                for _ in range(2)
            ]
            for g in range(4):
                for kb in (2 * g, 2 * g + 1):
                    for qc in range(2):
                        nc.tensor.matmul(
                            vp[qc][0:65, :],
                            vaug[kb][:, h * 65 : (h + 1) * 65],
                            pts[qc][:, kb, :],
                            start=(kb == 0),
                            stop=(kb == NB - 1),
                        )
                yield
            emit_pv_tail(h, vp)

        def emit_unit(d, qc, pv_gen):
            """Scores+exp for head pair (2d, 2d+1), one q-chunk, with the
            pending PV's matmul groups interleaved after each exp burst so
            neither PE nor ACT ever waits on the other. Even head on PE
            rows 0-63, odd head on rows 64-127 (disjoint row groups)."""
            qsl = slice(qc * QC, (qc + 1) * QC)
            ptA = sb.tile([P, NB, QC], bf16, tag="pt", bufs=8, name="ptA")
            ptB = sb.tile([P, NB, QC], bf16, tag="pt", bufs=8, name="ptB")
            for kb2 in range(NB // 2):
                spA = ps.tile([P, 2, QC], f32, tag="big", bufs=3, name="spA")
                spB = ps.tile([P, 2, QC], f32, tag="big", bufs=3, name="spB")
                for j in range(2):
                    kb = 2 * kb2 + j
                    ksl = slice(kb * P, (kb + 1) * P)
                    nc.tensor.matmul(
                        spA[:, j, :], kpt[d][0:DK, ksl], qpt[d][0:DK, qsl],
                        start=True, stop=True,
                    )
                    nc.tensor.matmul(
                        spB[:, j, :], kpt[d][DK:P, ksl], qpt[d][DK:P, qsl],
                        start=True, stop=True,
                    )
                nc.scalar.activation(
                    ptA[:, 2 * kb2 : 2 * kb2 + 2, :], spA, AF.Exp, scale=0.03125
                )
                nc.scalar.activation(
                    ptB[:, 2 * kb2 : 2 * kb2 + 2, :], spB, AF.Exp, scale=0.03125
                )
                if pv_gen is not None:
                    next(pv_gen, None)
            return ptA, ptB

        with nc.named_scope("attention"):
            # software-pipelined over head pairs; the previous pair's PV
            # matmuls ride inside the current scores unit:
            #   S(d,0)+PV_A(d-1), S(d,1)+PV_B(d-1), S(d+1,0)+PV_A(d), ...
            pend = {}
            prev = None
            for d in range(NB):
                g = (
                    gen_pv(2 * prev, [pend[(prev, 0)][0], pend[(prev, 1)][0]])
                    if prev is not None
                    else None
                )
                a0, b0 = emit_unit(d, 0, g)
                if g is not None:
                    for _ in g:
                        pass
                g = (
                    gen_pv(
                        2 * prev + 1, [pend[(prev, 0)][1], pend[(prev, 1)][1]]
                    )
                    if prev is not None
                    else None
                )
                a1, b1 = emit_unit(d, 1, g)
                if g is not None:
                    for _ in g:
                        pass
                    del pend[(prev, 0)], pend[(prev, 1)]
                pend[(d, 0)] = (a0, b0)
                pend[(d, 1)] = (a1, b1)
                prev = d
            for _ in gen_pv(2 * prev, [pend[(prev, 0)][0], pend[(prev, 1)][0]]):
                pass
            for _ in gen_pv(
                2 * prev + 1, [pend[(prev, 0)][1], pend[(prev, 1)][1]]
            ):
                pass

        # ---- output projection -----------------------------------------
        with nc.named_scope("o_proj"):
            wo = [[wload(WOd, db, c) for c in range(2)] for db in range(NB)]
            for sblk in range(NB):
                bigacc = ps.tile([P, 2, QC], f32, tag="big", bufs=3, name="oacc")
                acc = [bigacc[:, 0, :], bigacc[:, 1, :]]
                for db in range(NB):
                    for c in range(2):
                        nc.tensor.matmul(
                            acc[c],
                            ot[db][:, sblk * P : (sblk + 1) * P],
                            wo[db][c],
                            start=(db == 0),
                            stop=(db == NB - 1),
                        )
                for c in range(2):
                    if with_bo:
                        nc.vector.tensor_add(
                            acc[c], acc[c], bob[:, c * QC : (c + 1) * QC]
                        )
                    o = sb.tile([P, QC], f32, tag="obuf", bufs=3, name="obuf")
                    nc.scalar.activation(o, acc[c], AF.Relu)
                    nc.sync.dma_start(
                        outd[sblk * P : (sblk + 1) * P, c * QC : (c + 1) * QC], o
                    )

    _split_wide_waits(nc)
    return nc


_NC_CACHE = {}


def kernel(Q, K, V, WQ, bQ, WK, bK, WV, bV, WO, bO, h):
    Q, K, V = (np.ascontiguousarray(np.asarray(x, np.float32)) for x in (Q, K, V))
    WQ, WK, WV, WO = (
        np.ascontiguousarray(np.asarray(x, np.float32)) for x in (WQ, WK, WV, WO)
    )
    bQ, bK, bV, bO = (
        np.ascontiguousarray(np.asarray(x, np.float32)) for x in (bQ, bK, bV, bO)
    )
    h = int(np.asarray(h))
    assert h == H, f"kernel specialized for h=16, got {h}"
    B = Q.shape[0]
    assert Q.shape == (B, S, D) and B == N_CORES

    key = (bool(np.any(bV)), bool(np.any(bO)))
    if key not in _NC_CACHE:
        _NC_CACHE[key] = _build_nc(*key)
    nc = _NC_CACHE[key]

    in_maps = [
        {
            "Q": Q[b], "K": K[b], "V": V[b],
            "WQ": WQ, "WK": WK, "WV": WV, "WO": WO,
            "bQ": bQ, "bK": bK, "bV": bV, "bO": bO,
        }
        for b in range(B)
    ]
    trace = os.environ.get("BASS_MHA_TRACE") == "1"
    res = run_bass_kernel_spmd(
        nc, in_maps, core_ids=list(range(N_CORES)), trace=trace
    )
    if trace:
        kernel.last_results = res
    return np.stack([res.results[b]["out"] for b in range(B)], axis=0)
